# revision 44
# baseline (speedup 1.0000x reference)
"""Trainium2 Bass kernel for batched self-attention returning (out, attn).

Problem shapes (hardcoded): x [4, 4096, 256], Wq/Wk/Wv [256, 128],
bq/bk/bv [128].  out = softmax((x@Wq+bq)(x@Wk+bk)^T / sqrt(128)) @ (x@Wv+bv),
returns (out [4,4096,128], attn [4,4096,4096]).

Sharding: 8 cores = 4 batches x 2 query-halves (data parallel over B,
sequence-parallel over the query dim of the TxT score matrix).  Each core
computes K/V for its batch from x[b] and the scores/softmax/out rows for its
2048-query half.  No collectives; the host slices inputs and concatenates
outputs.

Per-core layout strategy: everything that feeds the PE array is kept
"transposed" ([d, t] with d on partitions) so projections and the score
matmuls need no data movement:
  qT/kT[d, t] = W^T x^T        (lhsT = W [d_in, d], rhs = x^T [d_in, t])
  scores[t, s] tile = qT_tile^T @ kT   (contraction over d on partitions)
Matmul operands are float32r (fp32 bits, single-pass PE streaming).
Softmax runs along the free dim with the row-sum fused into the exp
activation (accum_out); exp is written as bf16 (used only for attn@v and
the attn normalize; rel err stays ~3e-3).  attn = e * (1/rowsum) streams
to DRAM in 1024-wide chunks.  For attn @ v, 128x128 blocks of e are
PE-transposed (bf16, grouped 4 per PSUM tile) so the contraction over s
lands on partitions; out is rescaled by 1/rowsum after accumulation.
"""

import numpy as np

B, T, D_IN, D_MODEL = 4, 4096, 256, 128
N_CORES = 8
TQ = T // 2          # queries per core
P = 128              # partitions
NKC = D_IN // P      # contraction chunks for the projections (2)
NT = TQ // P         # query tiles per core (16)
SC = 512             # score chunk width
NS = T // SC         # score chunks per query tile (8)
NAV = T // P         # s-chunks for attn @ v (32)
SCALE = 1.0 / float(np.sqrt(np.float32(D_MODEL)))

_compiled = None


def _build():
    import concourse.bass as bass
    import concourse.mybir as mybir
    import concourse.tile as tile
    from concourse import bacc
    from concourse.bass import ts
    from concourse.masks import make_identity

    f32 = mybir.dt.float32
    f32r = mybir.dt.float32r
    AX = mybir.AxisListType
    ALU = mybir.AluOpType
    ACTF = mybir.ActivationFunctionType

    bf16 = mybir.dt.bfloat16
    nc = bacc.Bacc("TRN2", target_bir_lowering=False, debug=False)

    # float32r == fp32 bits; typing inputs as f32r end-to-end satisfies the
    # BIR verifier's "producer must round to FP32r" rule for fast PE matmuls.
    xT_d = nc.dram_tensor("xT", [D_IN, T], f32r, kind="ExternalInput").ap()
    xTq_d = nc.dram_tensor("xTq", [D_IN, TQ], f32r, kind="ExternalInput").ap()
    w_d = {
        n: nc.dram_tensor(n, [D_IN, D_MODEL], f32r, kind="ExternalInput").ap()
        for n in ("Wq", "Wk", "Wv")
    }
    b_d = {
        n: nc.dram_tensor(n, [D_MODEL, 1], f32, kind="ExternalInput").ap()
        for n in ("bq", "bk", "bv")
    }
    attn_d = nc.dram_tensor("attn", [TQ, T], f32, kind="ExternalOutput").ap()
    out_d = nc.dram_tensor("out", [TQ, D_MODEL], f32, kind="ExternalOutput").ap()

    with tile.TileContext(nc) as tc:
        with (
            tc.tile_pool(name="const", bufs=1) as cpool,
            tc.tile_pool(name="persist", bufs=1) as persist,
        ):
            ident_b = cpool.tile([P, P], bf16)
            make_identity(nc, ident_b[:])

            W = {}
            for n in ("Wq", "Wk", "Wv"):
                t_ = cpool.tile([P, NKC, D_MODEL], f32r, tag=f"W_{n}")
                for c in range(NKC):
                    nc.sync.dma_start(
                        out=t_[:, c, :], in_=w_d[n][c * P : (c + 1) * P, :]
                    )
                W[n] = t_
            bias = {}
            for n in ("bq", "bk", "bv"):
                t_ = cpool.tile([P, 1], f32, tag=f"b_{n}")
                nc.sync.dma_start(out=t_[:], in_=b_d[n])
                bias[n] = t_

            qT = persist.tile([P, TQ], bf16)      # [d, t] for this core's queries
            kT = persist.tile([P, T], bf16)       # [d, s]
            vS = persist.tile([P, NAV, D_MODEL], bf16)  # v chunks [s_local, c, d]

            # ---- projections ----
            with (
                tc.tile_pool(name="xin", bufs=1) as xin,
                tc.tile_pool(name="vtmp", bufs=1) as vtmp,
                tc.tile_pool(name="prps", bufs=2, space="PSUM") as prps,
            ):
                xT = xin.tile([P, NKC, T], f32r)
                xTq = xin.tile([P, NKC, TQ], f32r)
                # column-split input loads: lets the first projection chunks
                # (and thus tile-0 scores) start before the full x^T arrives
                for c in range(NKC):
                    for hcol in range(2):
                        nc.sync.dma_start(
                            out=xT[:, c, ts(hcol, T // 2)],
                            in_=xT_d[c * P : (c + 1) * P, ts(hcol, T // 2)],
                        )
                        nc.sync.dma_start(
                            out=xTq[:, c, ts(hcol, TQ // 2)],
                            in_=xTq_d[c * P : (c + 1) * P, ts(hcol, TQ // 2)],
                        )

                # qT = Wq^T @ xq^T (+bq), kT/vT likewise over the full seq
                def proj(dst, wname, bname, src, width, on_act=False,
                         chunks=None):
                    for i in (range(width // SC) if chunks is None else chunks):
                        ps = prps.tile([P, SC], f32)
                        for c in range(NKC):
                            nc.tensor.matmul(
                                ps[:],
                                lhsT=W[wname][:, c, :],
                                rhs=src[:, c, ts(i, SC)],
                                start=(c == 0),
                                stop=(c == NKC - 1),
                            )
                        if on_act:
                            nc.scalar.activation(
                                dst[:, ts(i, SC)],
                                ps[:],
                                ACTF.Identity,
                                bias=bias[bname][:],
                            )
                        else:
                            nc.vector.tensor_scalar_add(
                                dst[:, ts(i, SC)], ps[:], bias[bname][:]
                            )

                # tile-0 scores need only qT chunk 0 + kT; emit those first
                proj(qT, "Wq", "bq", xTq, TQ, chunks=[0])
                proj(kT, "Wk", "bk", xT, T)
                proj(qT, "Wq", "bq", xTq, TQ, chunks=[1, 2, 3])
                vT = vtmp.tile([P, T], bf16)
                proj(vT, "Wv", "bv", xT, T, on_act=True)

                # v chunks in [s, d] layout for the attn@v contraction
                for g in range(NAV // 4):
                    pt = prps.tile([P, 4, P], bf16, tag="vtp")
                    for j in range(4):
                        nc.tensor.transpose(
                            pt[:, j, :], vT[:, ts(g * 4 + j, P)], ident_b[:]
                        )
                    nc.vector.tensor_copy(vS[:, g * 4 : (g + 1) * 4, :], pt[:])

            # ---- main loop over query tiles ----
            with (
                tc.tile_pool(name="epool", bufs=4) as epool,
                tc.tile_pool(name="apool", bufs=8) as apool,
                tc.tile_pool(name="stat", bufs=4) as stat,
                tc.tile_pool(name="etp", bufs=12) as etp,
                tc.tile_pool(name="osb", bufs=6) as osbp,
                tc.tile_pool(name="scps", bufs=2, space="PSUM") as scps,
                tc.tile_pool(name="tpps", bufs=2, space="PSUM") as tpps,
                tc.tile_pool(name="outps", bufs=2, space="PSUM") as outps,
            ):
                EC = 1024          # exp chunk (2 psum banks)
                NE = T // EC       # 4 exp chunks / tile
                TG = 4             # transposed 128-blocks per PSUM group
                NG = NAV // TG     # 8 transpose groups / tile
                for tt in range(NT):
                    eb = epool.tile([P, T], bf16)
                    dsum = stat.tile([P, NE], f32)
                    for i in range(NE):
                        ps = scps.tile([P, EC], f32)
                        for half in range(EC // SC):
                            nc.tensor.matmul(
                                ps[:, ts(half, SC)],
                                lhsT=qT[:, ts(tt, P)],
                                rhs=kT[:, ts(i * 2 + half, SC)],
                                start=True,
                                stop=True,
                            )
                        nc.scalar.activation(
                            eb[:, ts(i, EC)],
                            ps[:],
                            ACTF.Exp,
                            scale=SCALE,
                            accum_out=dsum[:, i : i + 1],
                        )
                    den = stat.tile([P, 1], f32)
                    rden = stat.tile([P, 1], f32)
                    nc.vector.tensor_reduce(den[:], dsum[:], axis=AX.X, op=ALU.add)
                    nc.vector.reciprocal(rden[:], den[:])

                    outp = outps.tile([P, D_MODEL], f32)
                    for g in range(NG):
                        pt = tpps.tile([P, TG, P], bf16)
                        for j in range(TG):
                            nc.tensor.transpose(
                                pt[:, j, :], eb[:, ts(g * TG + j, P)], ident_b[:]
                            )
                        eT = etp.tile([P, TG, P], bf16)
                        nc.vector.tensor_copy(eT[:], pt[:])
                        for j in range(TG):
                            c = g * TG + j
                            nc.tensor.matmul(
                                outp[:],
                                lhsT=eT[:, j, :],
                                rhs=vS[:, c, :],
                                start=(c == 0),
                                stop=(c == NAV - 1),
                            )

                    for i in range(NE):
                        at = apool.tile([P, EC], f32)
                        nc.vector.tensor_scalar_mul(
                            at[:], eb[:, ts(i, EC)], rden[:]
                        )
                        nc.sync.dma_start(
                            out=attn_d[ts(tt, P), ts(i, EC)], in_=at[:]
                        )
                    ot = osbp.tile([P, D_MODEL], f32)
                    nc.vector.tensor_scalar_mul(ot[:], outp[:], rden[:])
                    nc.sync.dma_start(out=out_d[ts(tt, P), :], in_=ot[:])

    nc.compile()
    return nc


def kernel(x, Wq, bq, Wk, bk, Wv, bv):
    from concourse.bass_utils import run_bass_kernel_spmd

    global _compiled
    if _compiled is None:
        _compiled = _build()
    nc = _compiled

    x = np.ascontiguousarray(np.asarray(x, dtype=np.float32))
    ws = {n: np.ascontiguousarray(np.asarray(w, dtype=np.float32))
          for n, w in (("Wq", Wq), ("Wk", Wk), ("Wv", Wv))}
    bs = {n: np.ascontiguousarray(np.asarray(b, dtype=np.float32).reshape(D_MODEL, 1))
          for n, b in (("bq", bq), ("bk", bk), ("bv", bv))}

    in_maps = []
    for core in range(N_CORES):
        b, h = divmod(core, 2)
        xb = np.ascontiguousarray(x[b].T)                      # [256, 4096]
        xq = np.ascontiguousarray(x[b, h * TQ : (h + 1) * TQ].T)  # [256, 2048]
        in_maps.append({"xT": xb, "xTq": xq, **ws, **bs})

    res = None
    for attempt in range(3):
        try:
            res = run_bass_kernel_spmd(nc, in_maps, list(range(N_CORES)))
            break
        except Exception:
            if attempt == 2:
                raise
            import time

            time.sleep(5.0)

    out = np.empty((B, T, D_MODEL), dtype=np.float32)
    attn = np.empty((B, T, T), dtype=np.float32)
    for core in range(N_CORES):
        b, h = divmod(core, 2)
        out[b, h * TQ : (h + 1) * TQ] = res.results[core]["out"]
        attn[b, h * TQ : (h + 1) * TQ] = res.results[core]["attn"]
    return out, attn


# revision 45
# speedup vs baseline: 1.0074x; 1.0074x over previous
"""Trainium2 Bass kernel for batched self-attention returning (out, attn).

Problem shapes (hardcoded): x [4, 4096, 256], Wq/Wk/Wv [256, 128],
bq/bk/bv [128].  out = softmax((x@Wq+bq)(x@Wk+bk)^T / sqrt(128)) @ (x@Wv+bv),
returns (out [4,4096,128], attn [4,4096,4096]).

Sharding: 8 cores = 4 batches x 2 query-halves (data parallel over B,
sequence-parallel over the query dim of the TxT score matrix).  Each core
computes K/V for its batch from x[b] and the scores/softmax/out rows for its
2048-query half.  No collectives; the host slices inputs and concatenates
outputs.

Per-core layout strategy: everything that feeds the PE array is kept
"transposed" ([d, t] with d on partitions) so projections and the score
matmuls need no data movement:
  qT/kT[d, t] = W^T x^T        (lhsT = W [d_in, d], rhs = x^T [d_in, t])
  scores[t, s] tile = qT_tile^T @ kT   (contraction over d on partitions)
Matmul operands are float32r (fp32 bits, single-pass PE streaming).
Softmax runs along the free dim with the row-sum fused into the exp
activation (accum_out); exp is written as bf16 (used only for attn@v and
the attn normalize; rel err stays ~3e-3).  attn = e * (1/rowsum) streams
to DRAM in 1024-wide chunks.  For attn @ v, 128x128 blocks of e are
PE-transposed (bf16, grouped 4 per PSUM tile) so the contraction over s
lands on partitions; out is rescaled by 1/rowsum after accumulation.
"""

import numpy as np

B, T, D_IN, D_MODEL = 4, 4096, 256, 128
N_CORES = 8
TQ = T // 2          # queries per core
P = 128              # partitions
NKC = D_IN // P      # contraction chunks for the projections (2)
NT = TQ // P         # query tiles per core (16)
SC = 512             # score chunk width
NS = T // SC         # score chunks per query tile (8)
NAV = T // P         # s-chunks for attn @ v (32)
SCALE = 1.0 / float(np.sqrt(np.float32(D_MODEL)))

_compiled = None


def _build():
    import concourse.bass as bass
    import concourse.mybir as mybir
    import concourse.tile as tile
    from concourse import bacc
    from concourse.bass import ts
    from concourse.masks import make_identity

    f32 = mybir.dt.float32
    f32r = mybir.dt.float32r
    AX = mybir.AxisListType
    ALU = mybir.AluOpType
    ACTF = mybir.ActivationFunctionType

    bf16 = mybir.dt.bfloat16
    nc = bacc.Bacc("TRN2", target_bir_lowering=False, debug=False)

    # float32r == fp32 bits; typing inputs as f32r end-to-end satisfies the
    # BIR verifier's "producer must round to FP32r" rule for fast PE matmuls.
    xT_d = nc.dram_tensor("xT", [D_IN, T], f32r, kind="ExternalInput").ap()
    xTq_d = nc.dram_tensor("xTq", [D_IN, TQ], f32r, kind="ExternalInput").ap()
    w_d = {
        n: nc.dram_tensor(n, [D_IN, D_MODEL], f32r, kind="ExternalInput").ap()
        for n in ("Wq", "Wk", "Wv")
    }
    b_d = {
        n: nc.dram_tensor(n, [D_MODEL, 1], f32, kind="ExternalInput").ap()
        for n in ("bq", "bk", "bv")
    }
    attn_d = nc.dram_tensor("attn", [TQ, T], f32, kind="ExternalOutput").ap()
    out_d = nc.dram_tensor("out", [TQ, D_MODEL], f32, kind="ExternalOutput").ap()

    with tile.TileContext(nc) as tc:
        with (
            tc.tile_pool(name="const", bufs=1) as cpool,
            tc.tile_pool(name="persist", bufs=1) as persist,
        ):
            ident_b = cpool.tile([P, P], bf16)
            make_identity(nc, ident_b[:])

            W = {}
            for n in ("Wq", "Wk", "Wv"):
                t_ = cpool.tile([P, NKC, D_MODEL], f32r, tag=f"W_{n}")
                for c in range(NKC):
                    nc.sync.dma_start(
                        out=t_[:, c, :], in_=w_d[n][c * P : (c + 1) * P, :]
                    )
                W[n] = t_
            bias = {}
            for n in ("bq", "bk", "bv"):
                t_ = cpool.tile([P, 1], f32, tag=f"b_{n}")
                nc.sync.dma_start(out=t_[:], in_=b_d[n])
                bias[n] = t_

            qT = persist.tile([P, TQ], bf16)      # [d, t] for this core's queries
            kT = persist.tile([P, T], bf16)       # [d, s]
            vS = persist.tile([P, NAV, D_MODEL], bf16)  # v chunks [s_local, c, d]

            # ---- projections ----
            with (
                tc.tile_pool(name="xin", bufs=1) as xin,
                tc.tile_pool(name="vtmp", bufs=1) as vtmp,
                tc.tile_pool(name="prps", bufs=2, space="PSUM") as prps,
            ):
                xT = xin.tile([P, NKC, T], f32r)
                xTq = xin.tile([P, NKC, TQ], f32r)
                # column-split input loads: lets the first projection chunks
                # (and thus tile-0 scores) start before the full x^T arrives
                for c in range(NKC):
                    for hcol in range(2):
                        nc.sync.dma_start(
                            out=xT[:, c, ts(hcol, T // 2)],
                            in_=xT_d[c * P : (c + 1) * P, ts(hcol, T // 2)],
                        )
                        nc.sync.dma_start(
                            out=xTq[:, c, ts(hcol, TQ // 2)],
                            in_=xTq_d[c * P : (c + 1) * P, ts(hcol, TQ // 2)],
                        )

                # qT = Wq^T @ xq^T (+bq), kT/vT likewise over the full seq
                def proj(dst, wname, bname, src, width, on_act=False,
                         chunks=None):
                    for i in (range(width // SC) if chunks is None else chunks):
                        ps = prps.tile([P, SC], f32)
                        for c in range(NKC):
                            nc.tensor.matmul(
                                ps[:],
                                lhsT=W[wname][:, c, :],
                                rhs=src[:, c, ts(i, SC)],
                                start=(c == 0),
                                stop=(c == NKC - 1),
                            )
                        if on_act:
                            nc.scalar.activation(
                                dst[:, ts(i, SC)],
                                ps[:],
                                ACTF.Identity,
                                bias=bias[bname][:],
                            )
                        else:
                            nc.vector.tensor_scalar_add(
                                dst[:, ts(i, SC)], ps[:], bias[bname][:]
                            )

                # tile-0 scores need only qT chunk 0 + kT; emit those first
                proj(qT, "Wq", "bq", xTq, TQ, chunks=[0])
                proj(kT, "Wk", "bk", xT, T)
                proj(qT, "Wq", "bq", xTq, TQ, chunks=[1, 2, 3])
                vT = vtmp.tile([P, T], bf16)
                proj(vT, "Wv", "bv", xT, T, on_act=True)

                # v chunks in [s, d] layout for the attn@v contraction
                for g in range(NAV // 4):
                    pt = prps.tile([P, 4, P], bf16, tag="vtp")
                    for j in range(4):
                        nc.tensor.transpose(
                            pt[:, j, :], vT[:, ts(g * 4 + j, P)], ident_b[:]
                        )
                    nc.vector.tensor_copy(vS[:, g * 4 : (g + 1) * 4, :], pt[:])

            # ---- main loop over query tiles ----
            with (
                tc.tile_pool(name="epool", bufs=3) as epool,
                tc.tile_pool(name="apool", bufs=8) as apool,
                tc.tile_pool(name="stat", bufs=4) as stat,
                tc.tile_pool(name="etp", bufs=12) as etp,
                tc.tile_pool(name="osb", bufs=6) as osbp,
                tc.tile_pool(name="scps", bufs=2, space="PSUM") as scps,
                tc.tile_pool(name="tpps", bufs=2, space="PSUM") as tpps,
                tc.tile_pool(name="outps", bufs=2, space="PSUM") as outps,
            ):
                EC = 1024          # exp chunk (2 psum banks)
                NE = T // EC       # 4 exp chunks / tile
                TG = 4             # transposed 128-blocks per PSUM group
                NG = NAV // TG     # 8 transpose groups / tile
                for tt in range(NT):
                    eb = epool.tile([P, T], bf16)
                    dsum = stat.tile([P, NE], f32)
                    for i in range(NE):
                        ps = scps.tile([P, EC], f32)
                        for half in range(EC // SC):
                            nc.tensor.matmul(
                                ps[:, ts(half, SC)],
                                lhsT=qT[:, ts(tt, P)],
                                rhs=kT[:, ts(i * 2 + half, SC)],
                                start=True,
                                stop=True,
                            )
                        nc.scalar.activation(
                            eb[:, ts(i, EC)],
                            ps[:],
                            ACTF.Exp,
                            scale=SCALE,
                            accum_out=dsum[:, i : i + 1],
                        )
                    den = stat.tile([P, 1], f32)
                    rden = stat.tile([P, 1], f32)
                    nc.vector.tensor_reduce(den[:], dsum[:], axis=AX.X, op=ALU.add)
                    nc.vector.reciprocal(rden[:], den[:])

                    outp = outps.tile([P, D_MODEL], f32)
                    for g in range(NG):
                        pt = tpps.tile([P, TG, P], bf16)
                        for j in range(TG):
                            nc.tensor.transpose(
                                pt[:, j, :], eb[:, ts(g * TG + j, P)], ident_b[:]
                            )
                        eT = etp.tile([P, TG, P], bf16)
                        nc.vector.tensor_copy(eT[:], pt[:])
                        for j in range(TG):
                            c = g * TG + j
                            nc.tensor.matmul(
                                outp[:],
                                lhsT=eT[:, j, :],
                                rhs=vS[:, c, :],
                                start=(c == 0),
                                stop=(c == NAV - 1),
                            )

                    for i in range(NE):
                        at = apool.tile([P, EC], f32)
                        nc.vector.tensor_scalar_mul(
                            at[:], eb[:, ts(i, EC)], rden[:]
                        )
                        nc.sync.dma_start(
                            out=attn_d[ts(tt, P), ts(i, EC)], in_=at[:]
                        )
                    ot = osbp.tile([P, D_MODEL], f32)
                    nc.vector.tensor_scalar_mul(ot[:], outp[:], rden[:])
                    nc.sync.dma_start(out=out_d[ts(tt, P), :], in_=ot[:])

    nc.compile()
    return nc


def kernel(x, Wq, bq, Wk, bk, Wv, bv):
    from concourse.bass_utils import run_bass_kernel_spmd

    global _compiled
    if _compiled is None:
        _compiled = _build()
    nc = _compiled

    x = np.ascontiguousarray(np.asarray(x, dtype=np.float32))
    ws = {n: np.ascontiguousarray(np.asarray(w, dtype=np.float32))
          for n, w in (("Wq", Wq), ("Wk", Wk), ("Wv", Wv))}
    bs = {n: np.ascontiguousarray(np.asarray(b, dtype=np.float32).reshape(D_MODEL, 1))
          for n, b in (("bq", bq), ("bk", bk), ("bv", bv))}

    in_maps = []
    for core in range(N_CORES):
        b, h = divmod(core, 2)
        xb = np.ascontiguousarray(x[b].T)                      # [256, 4096]
        xq = np.ascontiguousarray(x[b, h * TQ : (h + 1) * TQ].T)  # [256, 2048]
        in_maps.append({"xT": xb, "xTq": xq, **ws, **bs})

    res = None
    for attempt in range(3):
        try:
            res = run_bass_kernel_spmd(nc, in_maps, list(range(N_CORES)))
            break
        except Exception:
            if attempt == 2:
                raise
            import time

            time.sleep(5.0)

    out = np.empty((B, T, D_MODEL), dtype=np.float32)
    attn = np.empty((B, T, T), dtype=np.float32)
    for core in range(N_CORES):
        b, h = divmod(core, 2)
        out[b, h * TQ : (h + 1) * TQ] = res.results[core]["out"]
        attn[b, h * TQ : (h + 1) * TQ] = res.results[core]["attn"]
    return out, attn
